# revision 29
# baseline (speedup 1.0000x reference)
"""Trainium2 Bass kernel for nn_JointSelfAttentionLayer.

Math restructuring (both outputs are sequence-means):
  C[b]    = (1/SC) * (colsum_b @ x_d[b]) @ W_vd,  colsum_b[t] = sum_s softmax(logits)[s,t]/sqrt(D)
  Dout[b] = (1/(SD*sqrt(D))) * (sum_s x_c[b,s,:]) @ W_vc   (softmax rows sum to 1)
so the only heavy device work is logits = x_c @ G @ x_d^T (G = W_qc @ W_kd^T)
plus a streaming softmax column-sum.

Precision plan (tolerance 2e-2; measured in numpy emulation 1.3e-3):
  - host computes G in fp32, ships G/x_c/x_d as f16 (10MB/core vs 32MB fp32)
  - device does single-pass f16 matmuls (1 cycle/row on the PE)
  - the tiny rank-1 epilogue products (u @ W_vd, xsum @ W_vc) run on host in fp32
x_c^T / x_d^T are produced by f16 PE transposes fed from plain DMA loads
(the xbar DMA-transpose path corrupts under multi-queue use and convoys
with concurrent copy DMAs, so it is avoided entirely).
"""
import numpy as np
from contextlib import ExitStack

B, SC, SD, D = 8, 2048, 2048, 1024
P = 128
DB = D // P            # 8 d-blocks
TB = SD // P           # 16 t-blocks
SBK = SC // P          # 16 s-blocks
CH = 512
NCH = SD // CH         # 4 chunks
INV_SQRT_D = 1.0 / 32.0


def _split_excess_waits(nc, mybir, max_waits=1):
    n = 0
    ctr = [0]
    for fn in nc.m.functions:
        for bb in fn.blocks:
            out = []
            changed = False
            for inst in bb.instructions:
                si = inst.sync_info
                ws = list(si.on_wait) if (si and si.on_wait) else []
                if len(ws) > max_waits and inst.engine != mybir.EngineType.Unassigned:
                    keep = ws[:max_waits]
                    excess = ws[max_waits:]
                    for i in range(0, len(excess), max_waits):
                        chunk = excess[i:i + max_waits]
                        nop = mybir.InstNoOp(name=f"ws_{ctr[0]}", ins=[], outs=[])
                        ctr[0] += 1
                        nop.engine = inst.engine
                        nop.sync_info = mybir.SyncInfo(on_wait=chunk, on_update=[])
                        out.append(nop)
                    inst.sync_info = mybir.SyncInfo(
                        on_wait=keep, on_update=list(si.on_update or []))
                    changed = True
                    n += 1
                out.append(inst)
            if changed:
                bb.instructions = out
    return n


def _build():
    import concourse.bass as bass
    import concourse.tile as tile
    from concourse import mybir
    from concourse.masks import make_identity

    F32 = mybir.dt.float32
    F16 = mybir.dt.float16
    Act = mybir.ActivationFunctionType
    Alu = mybir.AluOpType
    AxX = mybir.AxisListType.X

    nc = bass.Bass("TRN2", target_bir_lowering=False, debug=False, num_devices=8)
    xc = nc.dram_tensor("xc", [SC, D], F16, kind="ExternalInput").ap()
    xd = nc.dram_tensor("xd", [SD, D], F16, kind="ExternalInput").ap()
    g = nc.dram_tensor("g", [D, D], F16, kind="ExternalInput").ap()
    out_d = nc.dram_tensor("out", [P, 24], F32, kind="ExternalOutput").ap()

    with tile.TileContext(nc) as tc, ExitStack() as ctx:
        const = ctx.enter_context(tc.tile_pool(name="const", bufs=1))
        ident = const.tile([P, P], F32, name="ident")
        ident16 = const.tile([P, P], F16, name="ident16")
        cp = const.tile([P, SD], F32, name="cp")
        xsum = const.tile([P, DB], F32, name="xsum")
        out_sb = const.tile([P, 24], F32, name="out_sb")
        colsT = const.tile([P, TB], F32, name="colsT")

        big = ctx.enter_context(tc.tile_pool(name="big", bufs=1))
        gw = [big.tile([P, D], F16, name=f"g{i}") for i in range(DB)]
        xcT = [big.tile([P, SC], F16, name=f"xcT{j}") for j in range(DB)]
        xdT = [big.tile([P, SD], F16, name=f"xdT{j}") for j in range(DB)]
        ht = [big.tile([P, SC], F16, name=f"ht{j}") for j in range(DB)]

        # natural-layout x_d rows: used for the on-chip transpose AND the
        # phase-5 epilogue matmuls
        xdn = [big.tile([P, D], F16, name=f"xdn{t}") for t in range(TB)]

        # ---- loads (plain direct2d DMAs only; the xbar transpose path
        # convoys with concurrent copy DMAs, so transposes run on the PE).
        # x_c tiles gate the first transposes -> own gpsimd queue; G and x_d
        # ride the idle sync HWDGE queue.
        # identities first: ident16 gates the very first PE transpose and
        # affine_select runs on the same gpsimd queue as the x_c dispatches
        make_identity(nc, ident16[:])
        make_identity(nc, ident[:])
        xcn_pool = ctx.enter_context(tc.tile_pool(name="xcn", bufs=1))
        xcn = []
        for sb in range(SBK):
            t_ = xcn_pool.tile([P, D], F16, name=f"xcn{sb}", tag=f"xcn{sb % 8}")
            # first half on gpsimd (gates the first transposes), second half
            # rides sync so neither queue serializes 16-deep
            if sb < 8:
                nc.gpsimd.dma_start(t_[:], xc[sb * P:(sb + 1) * P, :])
            xcn.append(t_)
        for i in range(DB):
            nc.sync.dma_start(gw[i][:], g[i * P:(i + 1) * P, :])
        for sb in range(8, SBK):
            nc.sync.dma_start(xcn[sb][:], xc[sb * P:(sb + 1) * P, :])
        for t in range(TB):
            nc.sync.dma_start(xdn[t][:], xd[t * P:(t + 1) * P, :])
        nc.vector.memset(cp[:], 0.0)

        # ---- on-chip transposes + phase 2 ----
        with tc.tile_pool(name="tps", bufs=2, space="PSUM") as tps, \
             tc.tile_pool(name="p2ps", bufs=6, space="PSUM") as p2ps:
            # x_c^T: per (chunk c, dblock j): transpose 4 [128,128] blocks
            for c in range(SC // CH):
                for j in range(DB):
                    tp = tps.tile([P, CH], F16, name=f"tc{c}_{j}", tag="tp")
                    for q in range(4):
                        nc.tensor.transpose(tp[:, q * P:(q + 1) * P],
                                            xcn[c * 4 + q][:, j * P:(j + 1) * P],
                                            ident16[:])
                    nc.scalar.activation(xcT[j][:, c * CH:(c + 1) * CH], tp[:],
                                         Act.Copy)
            # x_d^T from the prefetched xdn tiles
            for c in range(SD // CH):
                for j in range(DB):
                    tp = tps.tile([P, CH], F16, name=f"td{c}_{j}", tag="tp")
                    for q in range(4):
                        nc.tensor.transpose(tp[:, q * P:(q + 1) * P],
                                            xdn[c * 4 + q][:, j * P:(j + 1) * P],
                                            ident16[:])
                    nc.scalar.activation(xdT[j][:, c * CH:(c + 1) * CH], tp[:],
                                         Act.Copy)

            # phase 2: HT[jp] = (x_c @ G)^T, f16 single pass
            for c in range(SC // CH):
                ssl = slice(c * CH, (c + 1) * CH)
                for jp in range(DB):
                    pg = p2ps.tile([P, CH], F32, name=f"pg{jp}_{c}", tag="pg")
                    for i in range(DB):
                        nc.tensor.matmul(pg[:], gw[i][:, jp * P:(jp + 1) * P],
                                         xcT[i][:, ssl],
                                         start=(i == 0), stop=(i == DB - 1))
                    nc.scalar.activation(ht[jp][:, ssl], pg[:], Act.Copy)

        # xsum[:, j] = sum_s x_c[s, j*128+p] (DVE; overlaps early phase 4)
        for j in range(DB):
            nc.vector.tensor_reduce(xsum[:, j:j + 1], xcT[j][:], AxX, Alu.add)

        # ---- phase 4: logits + softmax colsum ----
        with tc.tile_pool(name="p4", bufs=3) as p4, \
             tc.tile_pool(name="p4s", bufs=4) as p4s, \
             tc.tile_pool(name="p4ps", bufs=2, space="PSUM") as p4ps:
            for sb in range(SBK):
                L = p4ps.tile([P, SD], F32, name=f"L{sb}", tag="L")
                ssl = slice(sb * P, (sb + 1) * P)
                for c in range(NCH):
                    tsl = slice(c * CH, (c + 1) * CH)
                    for j in range(DB):
                        nc.tensor.matmul(L[:, tsl], ht[j][:, ssl], xdT[j][:, tsl],
                                         start=(j == 0), stop=(j == DB - 1))
                mx = p4s.tile([P, 1], F32, name=f"mx{sb}", tag="mx")
                nc.vector.tensor_reduce(mx[:], L[:], AxX, Alu.max)
                negmx = p4s.tile([P, 1], F32, name=f"negmx{sb}", tag="negmx")
                nc.vector.tensor_scalar_mul(negmx[:], mx[:], -1.0)
                E = p4.tile([P, SD], F16, name=f"E{sb}", tag="E")
                rs = p4s.tile([P, 1], F32, name=f"rs{sb}", tag="rs")
                nc.scalar.activation(E[:], L[:], Act.Exp,
                                     bias=negmx[:], scale=1.0, accum_out=rs[:])
                w = p4s.tile([P, 1], F32, name=f"w{sb}", tag="w")
                nc.vector.reciprocal(w[:], rs[:])
                w2 = p4s.tile([P, 1], F32, name=f"w2{sb}", tag="w2")
                nc.vector.tensor_scalar_mul(w2[:], w[:], INV_SQRT_D)
                # cp += E * w2 fused in one DVE pass
                nc.vector.scalar_tensor_tensor(cp[:], E[:], w2[:], cp[:],
                                               Alu.mult, Alu.add)

        # ---- phase 5: colsum out (u = colsum @ x_d runs on host in fp32) ----
        with tc.tile_pool(name="p5ps", bufs=6, space="PSUM") as p5ps:
            for t in range(TB):
                tp = p5ps.tile([P, P], F32, name=f"cpt{t}", tag="cpt")
                nc.tensor.transpose(tp[:], cp[:, t * P:(t + 1) * P], ident[:])
                nc.vector.tensor_reduce(colsT[:, t:t + 1], tp[:], AxX, Alu.add)
            nc.vector.tensor_copy(out_sb[:, 0:TB], colsT[:])
            nc.vector.tensor_copy(out_sb[:, TB:24], xsum[:])
            nc.scalar.dma_start(out_d[:], out_sb[:])

    _split_excess_waits(nc, mybir)
    return nc


def kernel(x_c, x_d, W_qc, W_vc, W_kd, W_vd):
    from concourse.bass_utils import run_bass_kernel_spmd
    f16 = np.float16
    W_qc = np.asarray(W_qc, dtype=np.float32)
    W_vc = np.asarray(W_vc, dtype=np.float32)
    W_kd = np.asarray(W_kd, dtype=np.float32)
    W_vd = np.asarray(W_vd, dtype=np.float32)
    g16 = (W_qc @ W_kd.T).astype(f16)
    xc16 = np.asarray(x_c).astype(f16)
    xd16 = np.asarray(x_d).astype(f16)

    nc = _build()
    in_maps = [{"xc": xc16[b], "xd": xd16[b], "g": g16} for b in range(B)]
    res = run_bass_kernel_spmd(nc, in_maps, list(range(B))).results

    colsum = np.empty((B, SD), dtype=np.float32)
    xs = np.empty((B, D), dtype=np.float32)
    for b in range(B):
        o = res[b]["out"]
        colsum[b] = o[:, :TB].T.ravel()
        xs[b] = o[:, TB:24].T.ravel()
    xd32 = np.asarray(x_d, dtype=np.float32)
    u = np.matmul(colsum[:, None, :], xd32)[:, 0, :]
    C = (u @ W_vd) / SC
    Dout = (xs @ W_vc) / (SD * 32.0)
    return (C, Dout)
